# revision 25
# baseline (speedup 1.0000x reference)
"""Trainium2 Bass kernel for nn_Attention (B=64, N=289, C=768, H=12).

Data-parallel over batch: 8 batches per NeuronCore x 8 cores, no collectives.

Per-core pipeline (all matmuls bf16 with f32 PSUM accumulation):
  1. qkv GEMM token-major: out[t_blk, f] = xT[c, t_blk].T @ wqkvT[c, f].
     Host appends per-head column sums of W so the GEMM also emits per-token
     head sums (the layernorm mean, which is linear in x).
  2. q/k head-wise layernorm: sum(q^2) via scalar_tensor_tensor accum_out,
     rsqrt via DVE bit-trick + Newton (keeps ScalarE exclusively on
     Exp/Copy/Identity -- no activation-table thrash), apply (q - m) * s as
     one tensor_scalar per head.
  3. rope fused with the LN gain/bias via host-precomputed tables
     (identity rotation rows for prefix/latent/pad tokens); even lanes on
     DVE, odd lanes on GPSIMD.  1/sqrt(D) folded into the q tables.
  4. q/k -> feature-major via XBAR DMA transpose of [128,128] bf16 blocks
     (two heads per 128-partition strip).
  5. scores computed transposed ST[k_tok, q_tok] (softmax without max
     subtraction -- LN-bounded logits), exp on ScalarE straight from PSUM.
  6. AV with a ones-column appended to V: OT[65, q] where row 64 carries the
     softmax sums; after all 12 heads of a batch, one batched reciprocal,
     partition-broadcast of the reciprocal row via a selector matmul on the
     PE, then normalize into bf16 OTn.
  7. proj GEMM feature-major + bias, DMA out as yT[768, 2312]; the host
     transposes back.
"""

import sys

if "/opt/trn_rl_repo" not in sys.path:
    sys.path.insert(0, "/opt/trn_rl_repo")

from contextlib import ExitStack

import ml_dtypes
import numpy as np

import concourse.bass as bass
import concourse.tile as tile
from concourse import bacc, mybir
from concourse.bass_utils import run_bass_kernel_spmd
from concourse.masks import make_identity

F32 = mybir.dt.float32
BF16 = mybir.dt.bfloat16
I32 = mybir.dt.int32
BF = ml_dtypes.bfloat16
OP = mybir.AluOpType
AF = mybir.ActivationFunctionType

B, N, C, H = 64, 289, 768, 12
D = C // H  # 64
NCORES = 8
BPC = B // NCORES  # 8 batches per core
T = BPC * N  # 2312 tokens per core
NT = (T + 127) // 128  # 19 token blocks
TPAD = NT * 128  # 2432
EPS = 1e-5
CHW = 6 * D + 6  # 390 columns per augmented qkv chunk (6 heads + 6 sums)
MAGIC = 0x5F3759DF

_CACHE = {}


def _batch_blocks():
    out, r = [], 0
    while r < N:
        rows = min(128, N - r)
        out.append((r, rows))
        r += rows
    return out


def _build_program():
    nc = bacc.Bacc("TRN2", target_bir_lowering=False, debug=False,
                   num_devices=NCORES)

    xT = nc.dram_tensor("xT", [C, TPAD], BF16, kind="ExternalInput").ap()
    wqkvT = nc.dram_tensor("wqkvT", [C, 6 * CHW], BF16,
                           kind="ExternalInput").ap()
    wprojT = nc.dram_tensor("wprojT", [C, C], BF16, kind="ExternalInput").ap()
    pbias = nc.dram_tensor("pbias", [C], F32, kind="ExternalInput").ap()
    # fused rope+LN tables, order: qCT, qST, qB2, kCT, kST, kB2
    tabs = nc.dram_tensor("tabs", [6, TPAD, D], BF16,
                          kind="ExternalInput").ap()
    sel = nc.dram_tensor("sel", [12, C], BF16, kind="ExternalInput").ap()
    out = nc.dram_tensor("out", [C, T], F32, kind="ExternalOutput").ap()

    with tile.TileContext(nc) as tc, ExitStack() as ctx:
        consts = ctx.enter_context(tc.tile_pool(name="consts", bufs=1))
        work = ctx.enter_context(tc.tile_pool(name="work", bufs=4))
        shal = ctx.enter_context(tc.tile_pool(name="shal", bufs=2))
        rope_scr = ctx.enter_context(tc.tile_pool(name="rope", bufs=2))
        strips = ctx.enter_context(tc.tile_pool(name="strips", bufs=1))
        vpool = ctx.enter_context(tc.tile_pool(name="vpool", bufs=6))
        ptpool = ctx.enter_context(tc.tile_pool(name="ptpool", bufs=6))
        otsbp = ctx.enter_context(tc.tile_pool(name="otsbp", bufs=7))
        otnpool = ctx.enter_context(tc.tile_pool(name="otnpool", bufs=13))
        bpool = ctx.enter_context(tc.tile_pool(name="bpool", bufs=2))
        mmps = ctx.enter_context(tc.tile_pool(name="mmps", bufs=3,
                                              space="PSUM"))
        scps = ctx.enter_context(tc.tile_pool(name="scps", bufs=2,
                                              space="PSUM"))
        otps = ctx.enter_context(tc.tile_pool(name="otps", bufs=1,
                                              space="PSUM"))

        # ---- persistent constants ----
        wq = []
        for c in range(6):
            t = consts.tile([128, 6 * CHW], BF16, tag=f"wq{c}")
            nc.sync.dma_start(t[:], wqkvT[c * 128:(c + 1) * 128, :])
            wq.append(t)
        wp = []
        for c in range(6):
            t = consts.tile([128, C], BF16, tag=f"wp{c}")
            nc.sync.dma_start(t[:], wprojT[c * 128:(c + 1) * 128, :])
            wp.append(t)
        pbias_t = consts.tile([128, 6], F32, tag="pbias")
        nc.sync.dma_start(pbias_t[:], pbias.rearrange("(a p) -> p a", p=128))
        sel_t = consts.tile([12, C], BF16, tag="sel")
        nc.sync.dma_start(sel_t[:], sel)
        magic_t = consts.tile([128, 12], I32, tag="magic")
        nc.vector.memset(magic_t[:], MAGIC)
        ident = consts.tile([128, 128], BF16, tag="ident")
        make_identity(nc, ident[:])
        tab_t = [[None] * NT for _ in range(6)]
        for k in range(6):
            for i in range(NT):
                t = consts.tile([128, D], BF16, tag=f"tab{k}_{i}")
                nc.sync.dma_start(t[:], tabs[k, i * 128:(i + 1) * 128, :])
                tab_t[k][i] = t

        # q/k feature-major strips: one per head pair, [128, TPAD]
        qT = [strips.tile([128, TPAD], BF16, tag=f"qT{p}", name=f"qT{p}")
              for p in range(6)]
        kT = [strips.tile([128, TPAD], BF16, tag=f"kT{p}", name=f"kT{p}")
              for p in range(6)]

        # ---- phase A: q,k GEMM + LN + rope + transpose, per 128-token blk
        # Transposes are software-pipelined: emitted TRANS_LAG halves after
        # their rot tile so the in-order PE queue never stalls on the
        # DVE/GPSIMD chain, with PSUM->SBUF copies split DVE/ACT and placed
        # at the head of each half's queue so the PSUM slots recycle fast.
        TRANS_LAG = 6
        pending = []  # (rot, dst, i)

        def flush_transposes(n):
            while len(pending) > n:
                rot_, dst_, i_ = pending.pop(0)
                flat = rot_[:].rearrange("p h d -> p (h d)")
                for p in range(6):
                    tp = scps.tile([128, 128], BF16, tag="sc", name="tp")
                    nc.tensor.transpose(
                        tp[:], flat[:, p * 128:(p + 1) * 128], ident[:])
                    nc.scalar.copy(
                        dst_[p][:, i_ * 128:(i_ + 1) * 128], tp[:])

        def emit_tile(i):
            xg = work.tile([128, 6, 128], BF16, tag="xg", name="xg")
            for c in range(6):
                nc.sync.dma_start(xg[:, c, :],
                                  xT[c * 128:(c + 1) * 128,
                                     i * 128:(i + 1) * 128])

            for half, (tb, rot_tag, dst) in enumerate(
                    ((0, "qrot", qT), (3, "krot", kT))):
                sums = work.tile([128, 12], F32, tag="sums")
                sumsq = work.tile([128, 12], F32, tag="sumsq")
                z = work.tile([128, H, D], BF16, tag="z")
                qsb = work.tile([128, H, D], BF16, tag="qsb")
                sq = shal.tile([128, H, D], BF16, tag="sq")
                for jl in range(2):
                    j = half * 2 + jl
                    ps = mmps.tile([128, CHW], F32, tag="mm")
                    for c in range(6):
                        nc.tensor.matmul(ps[:], lhsT=xg[:, c, :],
                                         rhs=wq[c][:, j * CHW:(j + 1) * CHW],
                                         start=(c == 0), stop=(c == 5))
                    nc.vector.tensor_scalar_mul(
                        sums[:, jl * 6:(jl + 1) * 6], ps[:, 6 * D:CHW],
                        1.0 / D)
                    # PSUM -> SBUF on ScalarE; DVE ops can read at most one
                    # PSUM operand.
                    nc.scalar.copy(
                        qsb[:, jl * 6:(jl + 1) * 6, :].rearrange(
                            "p h d -> p (h d)"), ps[:, :6 * D])
                flush_transposes(TRANS_LAG)
                nc.scalar.activation(sq[:], qsb[:], AF.Square)
                nc.vector.tensor_reduce(
                    out=sumsq[:], in_=sq[:],
                    axis=mybir.AxisListType.X, op=OP.add)

                # s = rsqrt(var + eps), var = sumsq/64 - mean^2
                m2 = shal.tile([128, 12], F32, tag="m2")
                vpe = shal.tile([128, 12], F32, tag="vpe")
                srs = work.tile([128, 12], F32, tag="srs")
                nt1 = shal.tile([128, 12], F32, tag="nt1")
                nc.vector.tensor_mul(m2[:], sums[:], sums[:])
                nc.vector.tensor_scalar(out=m2[:], in0=m2[:],
                                        scalar1=float(EPS), scalar2=None,
                                        op0=OP.subtract)
                nc.vector.tensor_scalar_mul(vpe[:], sumsq[:], 1.0 / D)
                nc.vector.tensor_sub(vpe[:], vpe[:], m2[:])
                nc.vector.tensor_scalar(out=srs[:].bitcast(I32),
                                        in0=vpe[:].bitcast(I32), scalar1=1,
                                        scalar2=None,
                                        op0=OP.logical_shift_right)
                nc.vector.tensor_tensor(out=srs[:].bitcast(I32),
                                        in0=magic_t[:],
                                        in1=srs[:].bitcast(I32),
                                        op=OP.subtract)
                for _ in range(1):  # Newton: y *= 1.5 - 0.5*x*y^2
                    nc.vector.tensor_mul(nt1[:], srs[:], srs[:])
                    nc.vector.tensor_mul(nt1[:], nt1[:], vpe[:])
                    nc.vector.tensor_scalar(out=nt1[:], in0=nt1[:],
                                            scalar1=-0.5, scalar2=1.5,
                                            op0=OP.mult, op1=OP.add)
                    nc.vector.tensor_mul(srs[:], srs[:], nt1[:])

                # z = (q - m) * s with stats broadcast along d
                nc.vector.tensor_tensor(
                    out=z[:], in0=qsb[:],
                    in1=sums[:, :, None].broadcast_to([128, H, D]),
                    op=OP.subtract)
                nc.vector.tensor_tensor(
                    out=z[:], in0=z[:],
                    in1=srs[:, :, None].broadcast_to([128, H, D]),
                    op=OP.mult)

                # rope (+ folded gain/bias), deinterleaved pair layout:
                # head cols [0:32] = even lanes, [32:64] = odd lanes
                # (host permuted the q/k weight columns to match).
                rot = work.tile([128, H, D], BF16, tag=rot_tag)
                zE, zO = z[:, :, 0:32], z[:, :, 32:64]
                CT, ST, B2 = (tab_t[tb][i], tab_t[tb + 1][i],
                              tab_t[tb + 2][i])

                def bc(ap):
                    return ap[:, None, :].broadcast_to([128, H, 32])

                a = rope_scr.tile([128, H, 32], BF16, tag="ra")
                b = rope_scr.tile([128, H, 32], BF16, tag="rb")
                nc.vector.tensor_mul(a[:], zE, bc(CT[:, 0:32]))
                nc.gpsimd.tensor_mul(b[:], zO, bc(ST[:, 0:32]))
                nc.vector.tensor_sub(a[:], a[:], b[:])
                nc.vector.tensor_add(rot[:, :, 0:32], a[:], bc(B2[:, 0:32]))
                c_ = rope_scr.tile([128, H, 32], BF16, tag="rc")
                d_ = rope_scr.tile([128, H, 32], BF16, tag="rd")
                nc.gpsimd.tensor_mul(c_[:], zO, bc(CT[:, 32:64]))
                nc.gpsimd.tensor_mul(d_[:], zE, bc(ST[:, 32:64]))
                nc.gpsimd.tensor_add(c_[:], c_[:], d_[:])
                nc.gpsimd.tensor_add(rot[:, :, 32:64], c_[:],
                                     bc(B2[:, 32:64]))
                pending.append((rot, dst, i))

        flush_transposes(0)

        # ---- phase B: per batch v GEMM, attention, proj ----
        kblocks = _batch_blocks()
        pending_proj = []

        def flush_proj(n):
            while len(pending_proj) > n + 1:
                otn_, b_ = pending_proj.pop(0)
                for co in range(6):
                    pp = scps.tile([128, 2, 512], F32, tag="sc", name="pp")
                    for cp in range(6):
                        nc.tensor.matmul(
                            pp[:, 0, :N],
                            lhsT=wp[cp][:, co * 128:(co + 1) * 128],
                            rhs=otn_[cp][:], start=(cp == 0), stop=(cp == 5))
                    ysb = bpool.tile([128, N], F32, tag="ysb", name="ysb")
                    nc.scalar.activation(ysb[:], pp[:, 0, :N], AF.Identity,
                                         bias=pbias_t[:, co:co + 1],
                                         scale=1.0)
                    nc.sync.dma_start(out[co * 128:(co + 1) * 128,
                                          b_ * N:(b_ + 1) * N], ysb[:])

        def emit_batch(b):
            v65 = []
            for (r0, rows) in kblocks:
                g0 = b * N + r0
                xb = work.tile([128, 6, 128], BF16, tag="xb")
                for c in range(6):
                    nc.sync.dma_start(xb[:, c, :rows],
                                      xT[c * 128:(c + 1) * 128, g0:g0 + rows])
                v = vpool.tile([128, H, D + 1], BF16, tag="v65")
                nc.vector.memset(v[:, :, D:D + 1], 1.0)
                for j in range(2):  # chunks 4,5 = v heads 0-5, 6-11
                    ps = mmps.tile([128, CHW], F32, tag="mm")
                    for c in range(6):
                        nc.tensor.matmul(
                            ps[:rows, :], lhsT=xb[:, c, :rows],
                            rhs=wq[c][:, (4 + j) * CHW:(5 + j) * CHW],
                            start=(c == 0), stop=(c == 5))
                    nc.vector.tensor_copy(
                        v[:rows, j * 6:(j + 1) * 6, :D],
                        ps[:rows, :6 * D].rearrange("p (h d) -> p h d", d=D))
                v65.append(v)

            # proj of the previous batch is emitted here so the in-order
            # PE queue never waits on the previous normalize chain.
            flush_proj(0)

            sums_sb = bpool.tile([12, N], F32, tag="sums_sb")
            otsb = [None] * 6
            for p in range(6):
                pts = []
                for (r0, rows) in kblocks:
                    sc = scps.tile([128, 2, 512], F32, tag="sc")
                    kc = b * N + r0
                    for h in range(2):
                        nc.tensor.matmul(
                            sc[:rows, h, :N],
                            lhsT=kT[p][h * D:(h + 1) * D, kc:kc + rows],
                            rhs=qT[p][h * D:(h + 1) * D, b * N:(b + 1) * N],
                            start=True, stop=True,
                            tile_position=(h * D, 0))
                    pt = ptpool.tile([128, 2, N], BF16, tag="pt")
                    nc.scalar.activation(pt[:rows, :, :], sc[:rows, :, :N],
                                         AF.Exp)
                    pts.append(pt)
                osb = otsbp.tile([128, N], BF16, tag="otsb")
                for h in range(2):
                    hh = 2 * p + h
                    ot = otps.tile([128, 512], F32, tag="ot")
                    for ik, (r0, rows) in enumerate(kblocks):
                        nc.tensor.matmul(
                            ot[:D + 1, :N], lhsT=v65[ik][:rows, hh, :],
                            rhs=pts[ik][:rows, h, :],
                            start=(ik == 0), stop=(ik == len(kblocks) - 1))
                    stmp = bpool.tile([1, N], F32, tag="stmp", name="stmp")
                    nc.vector.tensor_copy(stmp[:], ot[D:D + 1, :N])
                    nc.sync.dma_start(sums_sb[hh:hh + 1, :], stmp[:])
                    nc.vector.tensor_copy(osb[h * D:(h + 1) * D, :],
                                          ot[:D, :N])
                otsb[p] = osb

            rinv = bpool.tile([12, N], F32, tag="rinv")
            rinvb = bpool.tile([12, N], BF16, tag="rinvb")
            nc.vector.reciprocal(rinv[:], sums_sb[:])
            nc.vector.tensor_copy(rinvb[:], rinv[:])

            otn = []
            for p in range(6):
                o = otnpool.tile([128, N], BF16, tag="otn")
                # one matmul broadcasts both heads' reciprocal rows across
                # the pair's 128 partitions
                rb = otps.tile([128, 512], F32, tag="ot")
                nc.tensor.matmul(rb[:, :N],
                                 lhsT=sel_t[:, p * 128:(p + 1) * 128],
                                 rhs=rinvb[:], start=True, stop=True)
                nc.vector.tensor_mul(o[:], otsb[p][:], rb[:, :N])
                otn.append(o)
            pending_proj.append((otn, b))

        # interleave: emit each batch's attention as soon as its strip
        # columns are fully transposed, keeping PE (attention) and DVE
        # (LN/rope) simultaneously busy.
        def flush_through(tile_idx):
            while pending and pending[0][2] <= tile_idx:
                rot_, dst_, i_ = pending.pop(0)
                flat = rot_[:].rearrange("p h d -> p (h d)")
                for p in range(6):
                    tp = scps.tile([128, 128], BF16, tag="sc", name="tp")
                    nc.tensor.transpose(
                        tp[:], flat[:, p * 128:(p + 1) * 128], ident[:])
                    nc.scalar.copy(
                        dst_[p][:, i_ * 128:(i_ + 1) * 128], tp[:])

        emitted_b = 0
        for i in range(NT):
            emit_tile(i)
            while (emitted_b < BPC
                   and (N * (emitted_b + 1) + 127) // 128 - 1 <= i - 3):
                flush_through((N * (emitted_b + 1) + 127) // 128 - 1)
                emit_batch(emitted_b)
                emitted_b += 1
        flush_transposes(0)
        while emitted_b < BPC:
            emit_batch(emitted_b)
            emitted_b += 1
        flush_proj(-1)

    nc.compile()
    return nc


def _host_tables(rope_tensor, qn_g, qn_b, kn_g, kn_b, P, L):
    """Fused rope+LN tables [6, TPAD, 64]: qCT,qST,qB2,kCT,kST,kB2."""
    n_img = N - P - L
    rt = np.asarray(rope_tensor, np.float64)
    cos = rt[:n_img, :, 0]
    sin = rt[:n_img, :, 1]
    c_full = np.ones((N, D // 2))
    s_full = np.zeros((N, D // 2))
    c_full[P:N - L] = cos
    s_full[P:N - L] = sin
    reps = TPAD // N + 2
    c_all = np.tile(c_full, (reps, 1))[:TPAD]
    s_all = np.tile(s_full, (reps, 1))[:TPAD]
    c_all[T:] = 1.0
    s_all[T:] = 0.0

    def mk(g, b):
        # deinterleaved layout: cols [0:32] = even lanes, [32:64] = odd
        g = np.asarray(g, np.float64)
        b = np.asarray(b, np.float64)
        ge, go = g[0::2], g[1::2]
        be, bo = b[0::2], b[1::2]
        CT = np.empty((TPAD, D))
        ST = np.empty((TPAD, D))
        B2 = np.empty((TPAD, D))
        CT[:, 0:32] = ge[None, :] * c_all
        CT[:, 32:64] = go[None, :] * c_all
        ST[:, 0:32] = go[None, :] * s_all
        ST[:, 32:64] = ge[None, :] * s_all
        B2[:, 0:32] = be[None, :] * c_all - bo[None, :] * s_all
        B2[:, 32:64] = bo[None, :] * c_all + be[None, :] * s_all
        return CT, ST, B2

    qsc = 1.0 / np.sqrt(D)
    qCT, qST, qB2 = mk(np.asarray(qn_g, np.float64) * qsc,
                       np.asarray(qn_b, np.float64) * qsc)
    kCT, kST, kB2 = mk(kn_g, kn_b)
    return np.stack([qCT, qST, qB2, kCT, kST, kB2]).astype(BF)


def _host_wqkv(qkv_w):
    """wqkvT [C, 6*CHW]: 6 chunks of (6 heads x 64 cols + 6 head-sum cols).

    q/k head columns are permuted to the deinterleaved rope-pair layout
    ([evens, odds]); dot products over d are invariant since q and k get
    the same permutation.  v heads stay in natural order.
    """
    wT = np.asarray(qkv_w, np.float32).T  # [C, 3C]
    deint = np.concatenate([np.arange(0, D, 2), np.arange(1, D, 2)])
    outw = np.empty((C, 6 * CHW), np.float32)
    for j in range(6):
        cols = wT[:, j * 384:(j + 1) * 384].reshape(C, 6, D)
        sums = cols.sum(axis=2)
        if j < 4:  # q, k chunks: deinterleave each head's columns
            cols = cols[:, :, deint]
        outw[:, j * CHW:j * CHW + 384] = cols.reshape(C, 384)
        outw[:, j * CHW + 384:(j + 1) * CHW] = sums
    return outw.astype(BF)


def _host_sel():
    s = np.zeros((12, C), np.float32)
    for k in range(12):
        s[k, k * D:(k + 1) * D] = 1.0
    return s.astype(BF)


def _make_in_maps(x, rope_tensor, qkv_w, proj_w, proj_b, qn_g, qn_b,
                  kn_g, kn_b, P, L):
    tabs = _host_tables(rope_tensor, qn_g, qn_b, kn_g, kn_b, P, L)
    wqkvT = _host_wqkv(qkv_w)
    wprojT = np.ascontiguousarray(
        np.asarray(proj_w, np.float32).T).astype(BF)
    pb = np.ascontiguousarray(np.asarray(proj_b, np.float32))
    sel = _host_sel()
    in_maps = []
    for core in range(NCORES):
        xc = x[core * BPC:(core + 1) * BPC].reshape(T, C)
        xTc = np.zeros((C, TPAD), BF)
        xTc[:, :T] = xc.T.astype(BF)
        in_maps.append({"xT": xTc, "wqkvT": wqkvT, "wprojT": wprojT,
                        "pbias": pb, "tabs": tabs, "sel": sel})
    return in_maps


def kernel(x, rope_tensor, qkv_w, proj_w, proj_b, qn_g, qn_b, kn_g, kn_b,
           num_prefix_tokens, num_latent_tokens, _spmd_kwargs=None):
    P = int(num_prefix_tokens)
    L = int(num_latent_tokens)
    x = np.asarray(x, np.float32)
    assert x.shape == (B, N, C), x.shape

    if "nc" not in _CACHE:
        _CACHE["nc"] = _build_program()
    nc = _CACHE["nc"]

    in_maps = _make_in_maps(x, rope_tensor, qkv_w, proj_w, proj_b,
                            qn_g, qn_b, kn_g, kn_b, P, L)
    res = run_bass_kernel_spmd(nc, in_maps, core_ids=list(range(NCORES)),
                               **(_spmd_kwargs or {}))
    outs = []
    for core in range(NCORES):
        yT = np.asarray(res.results[core]["out"], np.float32)  # [C, T]
        outs.append(yT.T.reshape(BPC, N, C))
    full = np.concatenate(outs, axis=0).astype(np.float32)
    if _spmd_kwargs is not None:
        _CACHE["last_results"] = res
    return full


# revision 27
# speedup vs baseline: 1.0293x; 1.0293x over previous
"""Trainium2 Bass kernel for nn_Attention (B=64, N=289, C=768, H=12).

Data-parallel over batch: 8 batches per NeuronCore x 8 cores, no collectives.

Per-core pipeline (all matmuls bf16 with f32 PSUM accumulation):
  1. qkv GEMM token-major: out[t_blk, f] = xT[c, t_blk].T @ wqkvT[c, f].
     Host appends per-head column sums of W so the GEMM also emits per-token
     head sums (the layernorm mean, which is linear in x).
  2. q/k head-wise layernorm: sum(q^2) via scalar_tensor_tensor accum_out,
     rsqrt via DVE bit-trick + Newton (keeps ScalarE exclusively on
     Exp/Copy/Identity -- no activation-table thrash), apply (q - m) * s as
     one tensor_scalar per head.
  3. rope fused with the LN gain/bias via host-precomputed tables
     (identity rotation rows for prefix/latent/pad tokens); even lanes on
     DVE, odd lanes on GPSIMD.  1/sqrt(D) folded into the q tables.
  4. q/k -> feature-major via XBAR DMA transpose of [128,128] bf16 blocks
     (two heads per 128-partition strip).
  5. scores computed transposed ST[k_tok, q_tok] (softmax without max
     subtraction -- LN-bounded logits), exp on ScalarE straight from PSUM.
  6. AV with a ones-column appended to V: OT[65, q] where row 64 carries the
     softmax sums; after all 12 heads of a batch, one batched reciprocal,
     partition-broadcast of the reciprocal row via a selector matmul on the
     PE, then normalize into bf16 OTn.
  7. proj GEMM feature-major + bias, DMA out as yT[768, 2312]; the host
     transposes back.
"""

import sys

if "/opt/trn_rl_repo" not in sys.path:
    sys.path.insert(0, "/opt/trn_rl_repo")

from contextlib import ExitStack

import ml_dtypes
import numpy as np

import concourse.bass as bass
import concourse.tile as tile
from concourse import bacc, mybir
from concourse.bass_utils import run_bass_kernel_spmd
from concourse.masks import make_identity

F32 = mybir.dt.float32
BF16 = mybir.dt.bfloat16
I32 = mybir.dt.int32
BF = ml_dtypes.bfloat16
OP = mybir.AluOpType
AF = mybir.ActivationFunctionType

B, N, C, H = 64, 289, 768, 12
D = C // H  # 64
NCORES = 8
BPC = B // NCORES  # 8 batches per core
T = BPC * N  # 2312 tokens per core
NT = (T + 127) // 128  # 19 token blocks
TPAD = NT * 128  # 2432
EPS = 1e-5
CHW = 6 * D + 6  # 390 columns per augmented qkv chunk (6 heads + 6 sums)
MAGIC = 0x5F3759DF

_CACHE = {}


def _batch_blocks():
    out, r = [], 0
    while r < N:
        rows = min(128, N - r)
        out.append((r, rows))
        r += rows
    return out


def _build_program():
    nc = bacc.Bacc("TRN2", target_bir_lowering=False, debug=False,
                   num_devices=NCORES)

    xT = nc.dram_tensor("xT", [C, TPAD], BF16, kind="ExternalInput").ap()
    wqkvT = nc.dram_tensor("wqkvT", [C, 6 * CHW], BF16,
                           kind="ExternalInput").ap()
    wprojT = nc.dram_tensor("wprojT", [C, C], BF16, kind="ExternalInput").ap()
    pbias = nc.dram_tensor("pbias", [C], F32, kind="ExternalInput").ap()
    # fused rope+LN tables, order: qCT, qST, qB2, kCT, kST, kB2
    tabs = nc.dram_tensor("tabs", [6, TPAD, D], BF16,
                          kind="ExternalInput").ap()
    sel = nc.dram_tensor("sel", [12, C], BF16, kind="ExternalInput").ap()
    out = nc.dram_tensor("out", [C, T], F32, kind="ExternalOutput").ap()

    with tile.TileContext(nc) as tc, ExitStack() as ctx:
        consts = ctx.enter_context(tc.tile_pool(name="consts", bufs=1))
        work = ctx.enter_context(tc.tile_pool(name="work", bufs=4))
        shal = ctx.enter_context(tc.tile_pool(name="shal", bufs=2))
        rope_scr = ctx.enter_context(tc.tile_pool(name="rope", bufs=2))
        strips = ctx.enter_context(tc.tile_pool(name="strips", bufs=1))
        vpool = ctx.enter_context(tc.tile_pool(name="vpool", bufs=6))
        ptpool = ctx.enter_context(tc.tile_pool(name="ptpool", bufs=8))
        otsbp = ctx.enter_context(tc.tile_pool(name="otsbp", bufs=7))
        otnpool = ctx.enter_context(tc.tile_pool(name="otnpool", bufs=13))
        bpool = ctx.enter_context(tc.tile_pool(name="bpool", bufs=2))
        mmps = ctx.enter_context(tc.tile_pool(name="mmps", bufs=2,
                                              space="PSUM"))
        scps = ctx.enter_context(tc.tile_pool(name="scps", bufs=2,
                                              space="PSUM"))
        otps = ctx.enter_context(tc.tile_pool(name="otps", bufs=2,
                                              space="PSUM"))

        # ---- persistent constants ----
        wq = []
        for c in range(6):
            t = consts.tile([128, 6 * CHW], BF16, tag=f"wq{c}")
            nc.sync.dma_start(t[:], wqkvT[c * 128:(c + 1) * 128, :])
            wq.append(t)
        wp = []
        for c in range(6):
            t = consts.tile([128, C], BF16, tag=f"wp{c}")
            nc.sync.dma_start(t[:], wprojT[c * 128:(c + 1) * 128, :])
            wp.append(t)
        pbias_t = consts.tile([128, 6], F32, tag="pbias")
        nc.sync.dma_start(pbias_t[:], pbias.rearrange("(a p) -> p a", p=128))
        sel_t = consts.tile([12, C], BF16, tag="sel")
        nc.sync.dma_start(sel_t[:], sel)
        magic_t = consts.tile([128, 12], I32, tag="magic")
        nc.vector.memset(magic_t[:], MAGIC)
        ident = consts.tile([128, 128], BF16, tag="ident")
        make_identity(nc, ident[:])
        tab_t = [[None] * NT for _ in range(6)]
        for k in range(6):
            for i in range(NT):
                t = consts.tile([128, D], BF16, tag=f"tab{k}_{i}")
                nc.sync.dma_start(t[:], tabs[k, i * 128:(i + 1) * 128, :])
                tab_t[k][i] = t

        # q/k feature-major strips: one per head pair, [128, TPAD]
        qT = [strips.tile([128, TPAD], BF16, tag=f"qT{p}", name=f"qT{p}")
              for p in range(6)]
        kT = [strips.tile([128, TPAD], BF16, tag=f"kT{p}", name=f"kT{p}")
              for p in range(6)]

        # ---- phase A: q,k GEMM + LN + rope + transpose, per 128-token blk
        # Transposes are software-pipelined: emitted TRANS_LAG halves after
        # their rot tile so the in-order PE queue never stalls on the
        # DVE/GPSIMD chain, with PSUM->SBUF copies split DVE/ACT and placed
        # at the head of each half's queue so the PSUM slots recycle fast.
        TRANS_LAG = 6
        pending = []  # (rot, dst, i)

        def flush_transposes(n):
            while len(pending) > n:
                rot_, dst_, i_ = pending.pop(0)
                flat = rot_[:].rearrange("p h d -> p (h d)")
                for p in range(6):
                    tp = scps.tile([128, 128], BF16, tag="sc", name="tp")
                    nc.tensor.transpose(
                        tp[:], flat[:, p * 128:(p + 1) * 128], ident[:])
                    nc.scalar.copy(
                        dst_[p][:, i_ * 128:(i_ + 1) * 128], tp[:])

        def emit_tile(i):
            xg = work.tile([128, 6, 128], BF16, tag="xg", name="xg")
            for c in range(6):
                nc.sync.dma_start(xg[:, c, :],
                                  xT[c * 128:(c + 1) * 128,
                                     i * 128:(i + 1) * 128])

            for half, (tb, rot_tag, dst) in enumerate(
                    ((0, "qrot", qT), (3, "krot", kT))):
                sums = work.tile([128, 12], F32, tag="sums")
                sumsq = work.tile([128, 12], F32, tag="sumsq")
                z = work.tile([128, H, D], BF16, tag="z")
                qsb = work.tile([128, H, D], BF16, tag="qsb")
                sq = shal.tile([128, H, D], BF16, tag="sq")
                for jl in range(2):
                    j = half * 2 + jl
                    ps = mmps.tile([128, CHW], F32, tag="mm")
                    for c in range(6):
                        nc.tensor.matmul(ps[:], lhsT=xg[:, c, :],
                                         rhs=wq[c][:, j * CHW:(j + 1) * CHW],
                                         start=(c == 0), stop=(c == 5))
                    nc.vector.tensor_scalar_mul(
                        sums[:, jl * 6:(jl + 1) * 6], ps[:, 6 * D:CHW],
                        1.0 / D)
                    # PSUM -> SBUF on ScalarE; DVE ops can read at most one
                    # PSUM operand.
                    nc.scalar.copy(
                        qsb[:, jl * 6:(jl + 1) * 6, :].rearrange(
                            "p h d -> p (h d)"), ps[:, :6 * D])
                flush_transposes(TRANS_LAG)
                nc.scalar.activation(sq[:], qsb[:], AF.Square)
                nc.vector.tensor_reduce(
                    out=sumsq[:], in_=sq[:],
                    axis=mybir.AxisListType.X, op=OP.add)

                # s = rsqrt(var + eps), var = sumsq/64 - mean^2
                m2 = shal.tile([128, 12], F32, tag="m2")
                vpe = shal.tile([128, 12], F32, tag="vpe")
                srs = work.tile([128, 12], F32, tag="srs")
                nt1 = shal.tile([128, 12], F32, tag="nt1")
                nc.vector.tensor_mul(m2[:], sums[:], sums[:])
                nc.vector.tensor_scalar(out=m2[:], in0=m2[:],
                                        scalar1=float(EPS), scalar2=None,
                                        op0=OP.subtract)
                nc.vector.tensor_scalar_mul(vpe[:], sumsq[:], 1.0 / D)
                nc.vector.tensor_sub(vpe[:], vpe[:], m2[:])
                nc.vector.tensor_scalar(out=srs[:].bitcast(I32),
                                        in0=vpe[:].bitcast(I32), scalar1=1,
                                        scalar2=None,
                                        op0=OP.logical_shift_right)
                nc.vector.tensor_tensor(out=srs[:].bitcast(I32),
                                        in0=magic_t[:],
                                        in1=srs[:].bitcast(I32),
                                        op=OP.subtract)
                for _ in range(1):  # Newton: y *= 1.5 - 0.5*x*y^2
                    nc.vector.tensor_mul(nt1[:], srs[:], srs[:])
                    nc.vector.tensor_mul(nt1[:], nt1[:], vpe[:])
                    nc.vector.tensor_scalar(out=nt1[:], in0=nt1[:],
                                            scalar1=-0.5, scalar2=1.5,
                                            op0=OP.mult, op1=OP.add)
                    nc.vector.tensor_mul(srs[:], srs[:], nt1[:])

                # z = (q - m) * s with stats broadcast along d
                nc.vector.tensor_tensor(
                    out=z[:], in0=qsb[:],
                    in1=sums[:, :, None].broadcast_to([128, H, D]),
                    op=OP.subtract)
                nc.vector.tensor_tensor(
                    out=z[:], in0=z[:],
                    in1=srs[:, :, None].broadcast_to([128, H, D]),
                    op=OP.mult)

                # rope (+ folded gain/bias), deinterleaved pair layout:
                # head cols [0:32] = even lanes, [32:64] = odd lanes
                # (host permuted the q/k weight columns to match).
                rot = work.tile([128, H, D], BF16, tag=rot_tag)
                zE, zO = z[:, :, 0:32], z[:, :, 32:64]
                CT, ST, B2 = (tab_t[tb][i], tab_t[tb + 1][i],
                              tab_t[tb + 2][i])

                def bc(ap):
                    return ap[:, None, :].broadcast_to([128, H, 32])

                a = rope_scr.tile([128, H, 32], BF16, tag="ra")
                b = rope_scr.tile([128, H, 32], BF16, tag="rb")
                nc.vector.tensor_mul(a[:], zE, bc(CT[:, 0:32]))
                nc.gpsimd.tensor_mul(b[:], zO, bc(ST[:, 0:32]))
                nc.vector.tensor_sub(a[:], a[:], b[:])
                nc.vector.tensor_add(rot[:, :, 0:32], a[:], bc(B2[:, 0:32]))
                c_ = rope_scr.tile([128, H, 32], BF16, tag="rc")
                d_ = rope_scr.tile([128, H, 32], BF16, tag="rd")
                nc.gpsimd.tensor_mul(c_[:], zO, bc(CT[:, 32:64]))
                nc.gpsimd.tensor_mul(d_[:], zE, bc(ST[:, 32:64]))
                nc.gpsimd.tensor_add(c_[:], c_[:], d_[:])
                nc.gpsimd.tensor_add(rot[:, :, 32:64], c_[:],
                                     bc(B2[:, 32:64]))
                pending.append((rot, dst, i))

        flush_transposes(0)

        # ---- phase B: per batch v GEMM, attention, proj ----
        kblocks = _batch_blocks()
        pending_proj = []

        def flush_proj(n):
            while len(pending_proj) > n + 1:
                otn_, b_ = pending_proj.pop(0)
                for co in range(6):
                    pp = scps.tile([128, 2, 512], F32, tag="sc", name="pp")
                    for cp in range(6):
                        nc.tensor.matmul(
                            pp[:, 0, :N],
                            lhsT=wp[cp][:, co * 128:(co + 1) * 128],
                            rhs=otn_[cp][:], start=(cp == 0), stop=(cp == 5))
                    ysb = bpool.tile([128, N], F32, tag="ysb", name="ysb")
                    nc.scalar.activation(ysb[:], pp[:, 0, :N], AF.Identity,
                                         bias=pbias_t[:, co:co + 1],
                                         scale=1.0)
                    nc.sync.dma_start(out[co * 128:(co + 1) * 128,
                                          b_ * N:(b_ + 1) * N], ysb[:])

        def emit_batch(b):
            v65 = []
            for (r0, rows) in kblocks:
                g0 = b * N + r0
                xb = work.tile([128, 6, 128], BF16, tag="xb")
                for c in range(6):
                    nc.sync.dma_start(xb[:, c, :rows],
                                      xT[c * 128:(c + 1) * 128, g0:g0 + rows])
                v = vpool.tile([128, H, D + 1], BF16, tag="v65")
                nc.vector.memset(v[:, :, D:D + 1], 1.0)
                for j in range(2):  # chunks 4,5 = v heads 0-5, 6-11
                    ps = mmps.tile([128, CHW], F32, tag="mm")
                    for c in range(6):
                        nc.tensor.matmul(
                            ps[:rows, :], lhsT=xb[:, c, :rows],
                            rhs=wq[c][:, (4 + j) * CHW:(5 + j) * CHW],
                            start=(c == 0), stop=(c == 5))
                    nc.vector.tensor_copy(
                        v[:rows, j * 6:(j + 1) * 6, :D],
                        ps[:rows, :6 * D].rearrange("p (h d) -> p h d", d=D))
                v65.append(v)

            # proj of the previous batch is emitted here so the in-order
            # PE queue never waits on the previous normalize chain.
            flush_proj(0)

            sums_sb = bpool.tile([12, N], F32, tag="sums_sb")
            otsb = [None] * 6
            for p in range(6):
                pts = []
                for (r0, rows) in kblocks:
                    sc = scps.tile([128, 2, 512], F32, tag="sc")
                    kc = b * N + r0
                    for h in range(2):
                        nc.tensor.matmul(
                            sc[:rows, h, :N],
                            lhsT=kT[p][h * D:(h + 1) * D, kc:kc + rows],
                            rhs=qT[p][h * D:(h + 1) * D, b * N:(b + 1) * N],
                            start=True, stop=True,
                            tile_position=(h * D, 0))
                    pt = ptpool.tile([128, 2, N], BF16, tag="pt")
                    nc.scalar.activation(pt[:rows, :, :], sc[:rows, :, :N],
                                         AF.Exp)
                    pts.append(pt)
                osb = otsbp.tile([128, N], BF16, tag="otsb")
                for h in range(2):
                    hh = 2 * p + h
                    ot = otps.tile([128, 512], F32, tag="ot")
                    for ik, (r0, rows) in enumerate(kblocks):
                        nc.tensor.matmul(
                            ot[:D + 1, :N], lhsT=v65[ik][:rows, hh, :],
                            rhs=pts[ik][:rows, h, :],
                            start=(ik == 0), stop=(ik == len(kblocks) - 1))
                    stmp = bpool.tile([1, N], F32, tag="stmp", name="stmp")
                    nc.scalar.copy(stmp[:], ot[D:D + 1, :N])
                    nc.sync.dma_start(sums_sb[hh:hh + 1, :], stmp[:])
                    nc.vector.tensor_copy(osb[h * D:(h + 1) * D, :],
                                          ot[:D, :N])
                otsb[p] = osb

            rinv = bpool.tile([12, N], F32, tag="rinv")
            rinvb = bpool.tile([12, N], BF16, tag="rinvb")
            nc.vector.reciprocal(rinv[:], sums_sb[:])
            nc.vector.tensor_copy(rinvb[:], rinv[:])

            otn = []
            for p in range(6):
                o = otnpool.tile([128, N], BF16, tag="otn")
                # one matmul broadcasts both heads' reciprocal rows across
                # the pair's 128 partitions
                rb = otps.tile([128, 512], F32, tag="ot")
                nc.tensor.matmul(rb[:, :N],
                                 lhsT=sel_t[:, p * 128:(p + 1) * 128],
                                 rhs=rinvb[:], start=True, stop=True)
                nc.vector.tensor_mul(o[:], otsb[p][:], rb[:, :N])
                otn.append(o)
            pending_proj.append((otn, b))

        # interleave: emit each batch's attention as soon as its strip
        # columns are fully transposed, keeping PE (attention) and DVE
        # (LN/rope) simultaneously busy.
        def flush_through(tile_idx):
            while pending and pending[0][2] <= tile_idx:
                rot_, dst_, i_ = pending.pop(0)
                flat = rot_[:].rearrange("p h d -> p (h d)")
                for p in range(6):
                    tp = scps.tile([128, 128], BF16, tag="sc", name="tp")
                    nc.tensor.transpose(
                        tp[:], flat[:, p * 128:(p + 1) * 128], ident[:])
                    nc.scalar.copy(
                        dst_[p][:, i_ * 128:(i_ + 1) * 128], tp[:])

        emitted_b = 0
        for i in range(NT):
            emit_tile(i)
            while (emitted_b < BPC
                   and (N * (emitted_b + 1) + 127) // 128 - 1 <= i - 3):
                flush_through((N * (emitted_b + 1) + 127) // 128 - 1)
                emit_batch(emitted_b)
                emitted_b += 1
        flush_transposes(0)
        while emitted_b < BPC:
            emit_batch(emitted_b)
            emitted_b += 1
        flush_proj(-1)

    nc.compile()
    return nc


def _host_tables(rope_tensor, qn_g, qn_b, kn_g, kn_b, P, L):
    """Fused rope+LN tables [6, TPAD, 64]: qCT,qST,qB2,kCT,kST,kB2."""
    n_img = N - P - L
    rt = np.asarray(rope_tensor, np.float64)
    cos = rt[:n_img, :, 0]
    sin = rt[:n_img, :, 1]
    c_full = np.ones((N, D // 2))
    s_full = np.zeros((N, D // 2))
    c_full[P:N - L] = cos
    s_full[P:N - L] = sin
    reps = TPAD // N + 2
    c_all = np.tile(c_full, (reps, 1))[:TPAD]
    s_all = np.tile(s_full, (reps, 1))[:TPAD]
    c_all[T:] = 1.0
    s_all[T:] = 0.0

    def mk(g, b):
        # deinterleaved layout: cols [0:32] = even lanes, [32:64] = odd
        g = np.asarray(g, np.float64)
        b = np.asarray(b, np.float64)
        ge, go = g[0::2], g[1::2]
        be, bo = b[0::2], b[1::2]
        CT = np.empty((TPAD, D))
        ST = np.empty((TPAD, D))
        B2 = np.empty((TPAD, D))
        CT[:, 0:32] = ge[None, :] * c_all
        CT[:, 32:64] = go[None, :] * c_all
        ST[:, 0:32] = go[None, :] * s_all
        ST[:, 32:64] = ge[None, :] * s_all
        B2[:, 0:32] = be[None, :] * c_all - bo[None, :] * s_all
        B2[:, 32:64] = bo[None, :] * c_all + be[None, :] * s_all
        return CT, ST, B2

    qsc = 1.0 / np.sqrt(D)
    qCT, qST, qB2 = mk(np.asarray(qn_g, np.float64) * qsc,
                       np.asarray(qn_b, np.float64) * qsc)
    kCT, kST, kB2 = mk(kn_g, kn_b)
    return np.stack([qCT, qST, qB2, kCT, kST, kB2]).astype(BF)


def _host_wqkv(qkv_w):
    """wqkvT [C, 6*CHW]: 6 chunks of (6 heads x 64 cols + 6 head-sum cols).

    q/k head columns are permuted to the deinterleaved rope-pair layout
    ([evens, odds]); dot products over d are invariant since q and k get
    the same permutation.  v heads stay in natural order.
    """
    wT = np.asarray(qkv_w, np.float32).T  # [C, 3C]
    deint = np.concatenate([np.arange(0, D, 2), np.arange(1, D, 2)])
    outw = np.empty((C, 6 * CHW), np.float32)
    for j in range(6):
        cols = wT[:, j * 384:(j + 1) * 384].reshape(C, 6, D)
        sums = cols.sum(axis=2)
        if j < 4:  # q, k chunks: deinterleave each head's columns
            cols = cols[:, :, deint]
        outw[:, j * CHW:j * CHW + 384] = cols.reshape(C, 384)
        outw[:, j * CHW + 384:(j + 1) * CHW] = sums
    return outw.astype(BF)


def _host_sel():
    s = np.zeros((12, C), np.float32)
    for k in range(12):
        s[k, k * D:(k + 1) * D] = 1.0
    return s.astype(BF)


def _make_in_maps(x, rope_tensor, qkv_w, proj_w, proj_b, qn_g, qn_b,
                  kn_g, kn_b, P, L):
    tabs = _host_tables(rope_tensor, qn_g, qn_b, kn_g, kn_b, P, L)
    wqkvT = _host_wqkv(qkv_w)
    wprojT = np.ascontiguousarray(
        np.asarray(proj_w, np.float32).T).astype(BF)
    pb = np.ascontiguousarray(np.asarray(proj_b, np.float32))
    sel = _host_sel()
    in_maps = []
    for core in range(NCORES):
        xc = x[core * BPC:(core + 1) * BPC].reshape(T, C)
        xTc = np.zeros((C, TPAD), BF)
        xTc[:, :T] = xc.T.astype(BF)
        in_maps.append({"xT": xTc, "wqkvT": wqkvT, "wprojT": wprojT,
                        "pbias": pb, "tabs": tabs, "sel": sel})
    return in_maps


def kernel(x, rope_tensor, qkv_w, proj_w, proj_b, qn_g, qn_b, kn_g, kn_b,
           num_prefix_tokens, num_latent_tokens, _spmd_kwargs=None):
    P = int(num_prefix_tokens)
    L = int(num_latent_tokens)
    x = np.asarray(x, np.float32)
    assert x.shape == (B, N, C), x.shape

    if "nc" not in _CACHE:
        _CACHE["nc"] = _build_program()
    nc = _CACHE["nc"]

    in_maps = _make_in_maps(x, rope_tensor, qkv_w, proj_w, proj_b,
                            qn_g, qn_b, kn_g, kn_b, P, L)
    res = run_bass_kernel_spmd(nc, in_maps, core_ids=list(range(NCORES)),
                               **(_spmd_kwargs or {}))
    outs = []
    for core in range(NCORES):
        yT = np.asarray(res.results[core]["out"], np.float32)  # [C, T]
        outs.append(yT.T.reshape(BPC, N, C))
    full = np.concatenate(outs, axis=0).astype(np.float32)
    if _spmd_kwargs is not None:
        _CACHE["last_results"] = res
    return full
